# revision 1
# baseline (speedup 1.0000x reference)
"""LocallyConnected2d (64,64,32,32) x (1,64,64,32,32,9) -> (64,64,32,32) on 8 trn2 cores.

Strategy
--------
Spatial sharding over output rows: core i computes output rows [4i, 4i+4).

Per output location (x, y) the op is an independent GEMM:
    out[:, :, x, y] = patches(x,y) @ W(x,y).T + bias(:, x, y)
with contraction over (c, k) = 64*9 = 576, M = 64 out-channels, N = 64 batch.

On device, per location we issue 6 accumulating matmuls into PSUM:
  - x band lives in SBUF as [128, 64*204]: partitions 0-63 hold channels c
    (copy A), partitions 64-127 hold the same data shifted by +1 element
    (copy B), so a single K=128 matmul contracts over (c, two adjacent kernel
    taps) at once:
      chunk q in {0,1,2}: taps k=3q (copy A) and k=3q+1 (copy B), K=128
      single s in {0,1,2}: tap k=3s+2, K=64 (loc A on partitions 0-63,
      loc B on partitions 64-127 -- weights packed accordingly)
  - weights are host-prepacked to the exact [K, M] SBUF layout, streamed in
    8 blocks of 8 location-pairs.
  - bias is folded in with one K=8 indicator matmul per PSUM bank
    (psum[p, j*64+b] += bias_col_j[p] * ind[j, col]).

Outputs accumulate in PSUM banks of [128, 512] = 8 location-pairs, are
copied to SBUF by the vector engine and DMAed out in device-friendly
layout; the host untangles the layout at the end.

Compute dtype fp16 (fp32 accumulate in PSUM): 1 cycle/row on the PE vs 4
for fp32, and half the HBM traffic. |inputs| ~ N(0,1) so fp16 range is safe.
"""

import numpy as np

N_B, C, H, W_W, O = 64, 64, 32, 32, 64
KH = KW = 3
NCORES = 8
RPC = H // NCORES            # 4 output rows per core
BAND = RPC + 2               # 6 padded input rows per core
WP = W_W + 2                 # 34 padded width
XFREE = BAND * WP * N_B      # 13056, layout (h, w, b) -- b innermost
XPAD = 64                    # pad so the +1-w (=+64 elem) copy-B read is in bounds
NPAIR_CORE = RPC * W_W // 2  # 64 location pairs per core
NTILE = 8                    # PSUM tiles per core (8 pairs each)
PAIR_COLS = 576              # weight cols per location pair
W_FREE = NPAIR_CORE * PAIR_COLS  # 36864

COMPUTE_NP = np.float16      # np.float16 | np.float32 | ml_dtypes.bfloat16

_CACHE = {}


def _mybir_dt(np_dt):
    import concourse.mybir as mybir
    import ml_dtypes

    if np_dt == np.float16:
        return mybir.dt.float16
    if np_dt == np.float32:
        return mybir.dt.float32
    if np_dt == ml_dtypes.bfloat16:
        return mybir.dt.bfloat16
    raise ValueError(np_dt)


def build_nc(compute_np=None):
    """Build the (single-program) Bass kernel; same NEFF runs on all 8 cores."""
    import concourse.bass as bass  # noqa: F401
    import concourse.mybir as mybir
    import concourse.tile as tile
    from concourse import bacc
    from contextlib import ExitStack

    cdt = _mybir_dt(compute_np or COMPUTE_NP)
    f32 = mybir.dt.float32

    nc = bacc.Bacc("TRN2", target_bir_lowering=False, debug=False)

    x_dram = nc.dram_tensor("xb", [64, XFREE + XPAD], cdt, kind="ExternalInput")
    w_dram = nc.dram_tensor("wp", [128, W_FREE], cdt, kind="ExternalInput")
    b_dram = nc.dram_tensor("bp", [8, NTILE * 128], cdt, kind="ExternalInput")
    i_dram = nc.dram_tensor("ind", [8, 512], cdt, kind="ExternalInput")
    o_dram = nc.dram_tensor("out", [NTILE, 128, 512], f32, kind="ExternalOutput")

    with ExitStack() as ctx:
        tc = ctx.enter_context(tile.TileContext(nc))
        const = ctx.enter_context(tc.tile_pool(name="const", bufs=1))
        wpool = ctx.enter_context(tc.tile_pool(name="wpool", bufs=4))
        ppool = ctx.enter_context(tc.tile_pool(name="ppool", bufs=6, space="PSUM"))
        spool = ctx.enter_context(tc.tile_pool(name="spool", bufs=4))

        # x free layout: f = (h*34 + w)*64 + b -- batch innermost so matmul
        # rhs columns are contiguous (strided rhs measured 3x slower on PE).
        XH = 3 * WP * 64  # 6528, half the band (h rows 0-2)

        xsb = const.tile([128, XFREE], cdt)
        bias_sb = const.tile([8, NTILE * 128], cdt)
        ind_sb = const.tile([8, 512], cdt)
        # copy A (+0, partitions 0-63) on SP, copy B (+1 element, partitions
        # 64-127) on ACT: concurrent queues hit disjoint port halves. h-major
        # layout, chunked at h=3 so tile 0's rows land first.
        nc.sync.dma_start(xsb[0:64, 0:XH], x_dram.ap()[:, 0:XH])
        # copy B ( +1 w = +64 elems) built on-chip by the idle vector engine;
        # chunk ranges cover exactly the w<=32 reads each half needs.
        nc.vector.tensor_copy(xsb[64:128, 0 : XH - 64], xsb[0:64, 64:XH])
        nc.gpsimd.dma_start(bias_sb[:], b_dram.ap()[:, :])
        nc.gpsimd.dma_start(ind_sb[:], i_dram.ap()[:, :])

        x4 = xsb[:].rearrange("p (h w b) -> p h w b", h=BAND, w=WP)  # [128,6,34,64]

        for t in range(NTILE):
            wt = wpool.tile([128, 8 * PAIR_COLS], cdt)
            wbase = t * 8 * PAIR_COLS
            if t == 0:
                # split w0 so pair-0 matmuls can start before all 8 pairs land
                half = 4 * PAIR_COLS
                nc.sync.dma_start(
                    wt[:, 0:half], w_dram.ap()[:, wbase : wbase + half]
                )
                nc.sync.dma_start(
                    wt[:, half : 8 * PAIR_COLS],
                    w_dram.ap()[:, wbase + half : wbase + 8 * PAIR_COLS],
                )
                # second h-half of copy A after w0; copy B again via DVE
                nc.sync.dma_start(
                    xsb[0:64, XH:XFREE], x_dram.ap()[:, XH:XFREE]
                )
                nc.vector.tensor_copy(
                    xsb[64:128, XH - 64 : XFREE - 64], xsb[0:64, XH:XFREE]
                )
            else:
                weng = nc.sync if t % 2 == 0 else nc.scalar
                weng.dma_start(
                    wt[:], w_dram.ap()[:, wbase : wbase + 8 * PAIR_COLS]
                )
            ps = ppool.tile([128, 512], f32)
            xrow = t // 2
            for jp in range(8):
                jr = (t % 2) * 8 + jp       # pair index within the x-row
                yA = 2 * jr                 # w-offset of loc A
                base = jp * PAIR_COLS
                oc = jp * 64
                # loc A paired taps (k=3q copy A, k=3q+1 copy B), K=128
                for q in range(3):
                    nc.tensor.matmul(
                        ps[0:64, oc : oc + 64],
                        wt[:, base + q * 64 : base + (q + 1) * 64],
                        x4[:, xrow + q, yA, :],
                        start=(jp == 0 and q == 0),
                        stop=False,
                        skip_group_check=True,
                    )
                # loc B paired taps, K=128
                for q in range(3):
                    nc.tensor.matmul(
                        ps[64:128, oc : oc + 64],
                        wt[:, base + 192 + q * 64 : base + 192 + (q + 1) * 64],
                        x4[:, xrow + q, yA + 1, :],
                        start=(jp == 0 and q == 0),
                        stop=False,
                        skip_group_check=True,
                    )
                # single taps k=3s+2 (A rows 0-63 via copy A, B rows 64-127
                # via copy B), K=64 row-tiled
                for s in range(3):
                    sb = base + 384 + s * 64
                    nc.tensor.matmul(
                        ps[0:64, oc : oc + 64],
                        wt[0:64, sb : sb + 64],
                        x4[0:64, xrow + s, yA + 2, :],
                        start=False,
                        stop=False,
                        skip_group_check=True,
                    )
                    nc.tensor.matmul(
                        ps[64:128, oc : oc + 64],
                        wt[64:128, sb : sb + 64],
                        x4[64:128, xrow + s, yA + 2, :],
                        start=False,
                        stop=False,
                        skip_group_check=True,
                    )
            # bias: psum[p, j*64+b] += bias[j, t*128+p] * ind[j, col]
            nc.tensor.matmul(
                ps[:, :],
                bias_sb[:, t * 128 : (t + 1) * 128],
                ind_sb[:, :],
                start=False,
                stop=True,
                skip_group_check=True,
            )
            stg = spool.tile([128, 512], f32)
            nc.vector.tensor_copy(stg[:], ps[:])
            nc.sync.dma_start(o_dram.ap()[t], stg[:])

    nc.compile()
    return nc


def pack_inputs(x, weight, bias, compute_np=None):
    """Full fp32 inputs -> list of 8 per-core input dicts (device layouts)."""
    cnp = compute_np or COMPUTE_NP
    x = np.asarray(x)
    w5 = np.asarray(weight)[0]        # (o, c, x, y, k)
    b3 = np.asarray(bias)[0]          # (o, x, y)

    xp = np.pad(x, ((0, 0), (0, 0), (1, 1), (1, 1)))  # (b, c, 34, 34)

    ind = np.zeros((8, 512), dtype=cnp)
    for j in range(8):
        ind[j, j * 64 : (j + 1) * 64] = 1.0

    in_maps = []
    for i in range(NCORES):
        band = xp[:, :, RPC * i : RPC * i + BAND, :]          # (b, c, 6, 34)
        xb = np.ascontiguousarray(band.transpose(1, 2, 3, 0)) # (c, 6, 34, b)
        xb = xb.astype(cnp).reshape(64, XFREE)
        # trailing pad so the +64-element copy-B read stays in bounds
        xb = np.concatenate([xb, np.zeros((64, XPAD), dtype=cnp)], axis=1)

        wc = w5[:, :, RPC * i : RPC * (i + 1), :, :]          # (o, c, 4, 32, 9)
        wcr = wc.reshape(64, 64, 4, 16, 2, 9)                 # o c xh jr ab k
        chunks = wcr[..., [0, 1, 3, 4, 6, 7]].reshape(64, 64, 4, 16, 2, 3, 2)
        # -> [p=(half,c), j=(xh,jr), col=(ab,q,o)]
        chunks = chunks.transpose(6, 1, 2, 3, 4, 5, 0).reshape(128, 64, 384)
        singles = wcr[..., [2, 5, 8]]                         # o c xh jr ab s
        # -> [p=(ab,c), j=(xh,jr), col=(s,o)]
        singles = singles.transpose(4, 1, 2, 3, 5, 0).reshape(128, 64, 192)
        wp = np.concatenate([chunks, singles], axis=2)        # (128, 64, 576)
        wp = np.ascontiguousarray(wp).astype(cnp).reshape(128, W_FREE)

        bc = b3[:, RPC * i : RPC * (i + 1), :]                # (o, 4, 32)
        bcr = bc.reshape(64, 4, 2, 8, 2)                      # o xh half j' hi
        bp = bcr.transpose(3, 1, 2, 4, 0).reshape(8, NTILE * 128).astype(cnp)

        in_maps.append(
            {
                "xb": xb,
                "wp": wp,
                "bp": np.ascontiguousarray(bp),
                "ind": ind,
            }
        )
    return in_maps


def unpack_output(core_outs):
    """8 per-core [NTILE,128,512] arrays -> full (64, 64, 32, 32) output."""
    arr = np.stack(core_outs)                     # (core, t, p, col)
    arr = arr.reshape(8, 4, 2, 2, 64, 8, 64)      # core xh half hi o j' b
    out = arr.transpose(6, 4, 0, 1, 2, 5, 3)      # b o core xh half j' hi
    return np.ascontiguousarray(out.reshape(64, 64, 32, 32), dtype=np.float32)


def run_on_device(in_maps, trace=False, compute_np=None, **kwargs):
    from concourse import bass_utils

    key = ("nc", np.dtype(compute_np or COMPUTE_NP).name)
    if key not in _CACHE:
        _CACHE[key] = build_nc(compute_np)
    nc = _CACHE[key]
    res = bass_utils.run_bass_kernel_spmd(
        nc, in_maps, core_ids=list(range(NCORES)), trace=trace, **kwargs
    )
    return res


def kernel(x, weight, bias):
    in_maps = pack_inputs(x, weight, bias)
    res = run_on_device(in_maps)
    return unpack_output([r["out"] for r in res.results])



# revision 4
# speedup vs baseline: 1.1618x; 1.1618x over previous
"""LocallyConnected2d (64,64,32,32) x (1,64,64,32,32,9) -> (64,64,32,32) on 8 trn2 cores.

Strategy
--------
Spatial sharding over output rows: core i computes output rows [4i, 4i+4).

Per output location the op is an independent tiny GEMM contracting over
(c, k) = 64*9 = 576 with M = 64 out-channels, N = 64 batch. Adjacent
output columns (A=2u, B=2u+1) share two of their three input columns, so
we pack each pair into dense full-width matmuls:

  - shared taps: per input row r, one M=128 K=128 matmul.
    stationary [k=(c, col 2u+1 | c, col 2u+2), m=(A o | B o)] is 100%
    dense (A's kw1/kw2 and B's kw0/kw1); the moving operand is the
    (copy-A | copy-B) x layout read at base col 2u+1. M=128 stationaries
    are FWL-eligible (2x weight load vs the M=64 form).
  - exclusive taps (A kw0 at col 2u, B kw2 at col 2u+3): per input row,
    two concurrent M=64 K=64 col-group matmuls (tile_position (0,0) and
    (64,64)), reading copy-A at col 2u and copy-B at col 2u+2.
  - pairs at the image edge skip the exclusive matmul that would read
    the zero padding.

This cuts LDWEIGHTS columns 1.5x and matmul count 776 -> ~580 vs the
previous per-tap M=64 scheme, pushing the PE stream under the DMA wall.

DMA: all weights are SBUF-resident (16 half-bank chunks, no pool
recycling), issued upfront alternating sync/scalar HWDGE queues so the
weight stream runs at full rate instead of being throttled by PE
progress. x lower half on sync first; copy-B halves built by the DVE.
Outputs staged as fp16 (halves write traffic) and DMAed on the gpsimd
queue so they never queue behind weights. Bias is folded in with one
K=8 indicator matmul per PSUM bank. Host unpacks the device layout.

Compute dtype fp16 (fp32 accumulate in PSUM): |inputs| ~ N(0,1) so fp16
range is safe; measured rel err ~3e-4 vs fp32 reference.
"""

import numpy as np

N_B, C, H, W_W, O = 64, 64, 32, 32, 64
KH = KW = 3
NCORES = 8
RPC = H // NCORES            # 4 output rows per core
BAND = RPC + 2               # 6 padded input rows per core
WP = W_W + 2                 # 34 padded width
XFREE = BAND * WP * N_B      # 13056, layout (h, w, b) -- b innermost
XPAD = 64                    # pad so the +1-col (=+64 elem) copy-B read is in bounds
NTILE = 8                    # PSUM banks per core (8 pairs each)
PAIR_COLS = 576              # weight cols per location pair (384 shared + 192 excl)
W_FREE = NTILE * 8 * PAIR_COLS  # 36864
XH = 3 * WP * 64             # 6528: first chunk = padded rows 0-2

COMPUTE_NP = np.float16

_CACHE = {}


def _mybir_dt(np_dt):
    import concourse.mybir as mybir
    import ml_dtypes

    if np_dt == np.float16:
        return mybir.dt.float16
    if np_dt == np.float32:
        return mybir.dt.float32
    if np_dt == ml_dtypes.bfloat16:
        return mybir.dt.bfloat16
    raise ValueError(np_dt)


def build_nc(compute_np=None):
    """Build the (single-program) Bass kernel; same NEFF runs on all 8 cores."""
    import concourse.bass as bass  # noqa: F401
    import concourse.mybir as mybir
    import concourse.tile as tile
    from concourse import bacc
    from contextlib import ExitStack

    cdt = _mybir_dt(compute_np or COMPUTE_NP)
    f32 = mybir.dt.float32

    nc = bacc.Bacc("TRN2", target_bir_lowering=False, debug=False)

    x_dram = nc.dram_tensor("xb", [64, XFREE + XPAD], cdt, kind="ExternalInput")
    w_dram = nc.dram_tensor("wp", [128, W_FREE], cdt, kind="ExternalInput")
    b_dram = nc.dram_tensor("bp", [8, NTILE * 128], cdt, kind="ExternalInput")
    i_dram = nc.dram_tensor("ind", [8, 512], cdt, kind="ExternalInput")
    o_dram = nc.dram_tensor("out", [NTILE, 128, 512], cdt, kind="ExternalOutput")

    HALF_COLS = 4 * PAIR_COLS  # 2304 cols per half-bank DMA chunk

    with ExitStack() as ctx:
        tc = ctx.enter_context(tile.TileContext(nc))
        const = ctx.enter_context(tc.tile_pool(name="const", bufs=1))
        wpool = ctx.enter_context(tc.tile_pool(name="wpool", bufs=16))
        ppool = ctx.enter_context(tc.tile_pool(name="ppool", bufs=8, space="PSUM"))
        spool = ctx.enter_context(tc.tile_pool(name="spool", bufs=8))

        xsb = const.tile([128, XFREE], cdt)
        bias_sb = const.tile([8, NTILE * 128], cdt)
        ind_sb = const.tile([8, 512], cdt)

        # x rows 0-2 lower half first on sync (HWDGE, gates the first matmuls)
        nc.sync.dma_start(xsb[0:64, 0:XH], x_dram.ap()[:, 0:XH])
        # copy-B (+1 col = +64 elems) upper halves built by the otherwise-idle DVE
        nc.vector.tensor_copy(xsb[64:128, 0 : XH - 64], xsb[0:64, 64:XH])
        # x rows 3-5 + bias + indicator on the gpsimd queue
        nc.gpsimd.dma_start(xsb[0:64, XH:XFREE], x_dram.ap()[:, XH:XFREE])
        nc.vector.tensor_copy(xsb[64:128, XH - 64 : XFREE - 64], xsb[0:64, XH:XFREE])
        nc.gpsimd.dma_start(bias_sb[:], b_dram.ap()[:, :])
        nc.gpsimd.dma_start(ind_sb[:], i_dram.ap()[:, :])

        # all 16 weight half-bank chunks upfront, alternating the two HWDGE rings
        wts = []
        for t in range(NTILE):
            for h in range(2):
                wt = wpool.tile([128, HALF_COLS], cdt)
                off = (t * 2 + h) * HALF_COLS
                eng = nc.sync if h == 0 else nc.scalar
                eng.dma_start(wt[:], w_dram.ap()[:, off : off + HALF_COLS])
                wts.append(wt)

        for t in range(NTILE):
            x_row = t // 2
            ps = ppool.tile([128, 512], f32)
            first = True
            for jp in range(8):
                wt = wts[t * 2 + (jp // 4)]
                base = (jp % 4) * PAIR_COLS
                u = (t % 2) * 8 + jp          # global pair index in this x-row
                oc = jp * 64
                # shared taps: M=128 K=128, moving = (copyA|copyB) at col 2u+1
                for r in range(3):
                    fo = ((x_row + r) * WP + 2 * u + 1) * 64
                    nc.tensor.matmul(
                        ps[:, oc : oc + 64],
                        wt[:, base + r * 128 : base + (r + 1) * 128],
                        xsb[:, fo : fo + 64],
                        start=first,
                        stop=False,
                        skip_group_check=True,
                    )
                    first = False
                # exclusive taps: concurrent M=64 K=64 col-group matmuls
                for r in range(3):
                    eb = base + 384 + r * 64
                    if u > 0:  # A kw0 reads col 2u (zero pad when u==0)
                        foA = ((x_row + r) * WP + 2 * u) * 64
                        nc.tensor.matmul(
                            ps[0:64, oc : oc + 64],
                            wt[0:64, eb : eb + 64],
                            xsb[0:64, foA : foA + 64],
                            start=False,
                            stop=False,
                            skip_group_check=True,
                        )
                    if u < 15:  # B kw2 reads col 2u+3 (zero pad when u==15)
                        foB = ((x_row + r) * WP + 2 * u + 2) * 64
                        nc.tensor.matmul(
                            ps[64:128, oc : oc + 64],
                            wt[64:128, eb : eb + 64],
                            xsb[64:128, foB : foB + 64],
                            start=False,
                            stop=False,
                            skip_group_check=True,
                        )
            # bias: psum[p, j*64+b] += bias[j, t*128+p] * ind[j, col]
            nc.tensor.matmul(
                ps[:, :],
                bias_sb[:, t * 128 : (t + 1) * 128],
                ind_sb[:, :],
                start=False,
                stop=True,
                skip_group_check=True,
            )
            stg = spool.tile([128, 512], cdt)
            nc.scalar.copy(stg[:], ps[:])
            nc.gpsimd.dma_start(o_dram.ap()[t], stg[:])

    nc.compile()
    return nc


def pack_inputs(x, weight, bias, compute_np=None):
    """Full fp32 inputs -> list of 8 per-core input dicts (device layouts)."""
    cnp = compute_np or COMPUTE_NP
    x = np.asarray(x)
    w5 = np.asarray(weight)[0]        # (o, c, X, Y, k)
    b3 = np.asarray(bias)[0]          # (o, X, Y)

    xp = np.pad(x, ((0, 0), (0, 0), (1, 1), (1, 1)))  # (b, c, 34, 34)

    ind = np.zeros((8, 512), dtype=cnp)
    for j in range(8):
        ind[j, j * 64 : (j + 1) * 64] = 1.0

    in_maps = []
    for i in range(NCORES):
        band = xp[:, :, RPC * i : RPC * i + BAND, :]          # (b, c, 6, 34)
        xb = np.ascontiguousarray(band.transpose(1, 2, 3, 0)) # (c, 6, 34, b)
        xb = xb.astype(cnp).reshape(64, XFREE)
        xb = np.concatenate([xb, np.zeros((64, XPAD), dtype=cnp)], axis=1)

        wc = w5[:, :, RPC * i : RPC * (i + 1), :, :]          # (o, c, 4, 32, 9)
        # (x, u, c, ab, k, o)
        wt_ = wc.reshape(64, 64, 4, 16, 2, 9).transpose(2, 3, 1, 4, 5, 0)
        # shared stationary: [x, u, r, p2(c-half: col 2u+1 / 2u+2), c, m2(A/B), o]
        # col 2u+1 is A kw1 / B kw0; col 2u+2 is A kw2 / B kw1
        KWT = ((1, 0), (2, 1))
        S = np.empty((4, 16, 3, 2, 64, 2, 64), dtype=np.float32)
        for r in range(3):
            for p2 in range(2):
                for m2 in range(2):
                    S[:, :, r, p2, :, m2, :] = wt_[:, :, :, m2, 3 * r + KWT[p2][m2], :]
        # exclusive stationary: [x, u, r, half(eA/eB), c, o]
        E = np.empty((4, 16, 3, 2, 64, 64), dtype=np.float32)
        for r in range(3):
            E[:, :, r, 0] = wt_[:, :, :, 0, 3 * r + 0, :]     # A kw0
            E[:, :, r, 1] = wt_[:, :, :, 1, 3 * r + 2, :]     # B kw2
        # per pair: cols = [r, m2, o] (384 shared) then [r, o] (192 excl),
        # partitions = (p2|half, c)
        Sp = S.transpose(0, 1, 3, 4, 2, 5, 6).reshape(4, 16, 128, 384)
        Ep = E.transpose(0, 1, 3, 4, 2, 5).reshape(4, 16, 128, 192)
        wq = np.concatenate([Sp, Ep], axis=3).reshape(4, 2, 8, 128, PAIR_COLS)
        wq = wq.transpose(3, 0, 1, 2, 4)                      # (p, x, ugrp, jp, cols)
        wq = np.ascontiguousarray(wq).astype(cnp).reshape(128, W_FREE)

        bc = b3[:, RPC * i : RPC * (i + 1), :]                # (o, 4, 32)
        bcr = bc.reshape(64, 4, 16, 2)                        # o x u half
        bcr = bcr.reshape(64, 4, 2, 8, 2)                     # o x ugrp j half
        # bp[j, (x, ugrp), half, o]
        bp = bcr.transpose(3, 1, 2, 4, 0).reshape(8, NTILE * 128).astype(cnp)

        in_maps.append(
            {
                "xb": xb,
                "wp": wq,
                "bp": np.ascontiguousarray(bp),
                "ind": ind,
            }
        )
    return in_maps


def unpack_output(core_outs):
    """8 per-core [NTILE,128,512] arrays -> full (64, 64, 32, 32) output."""
    arr = np.stack([np.asarray(a, dtype=np.float32) for a in core_outs])
    arr = arr.reshape(8, 4, 2, 2, 64, 8, 64)      # core x ugrp half o jp b
    out = arr.transpose(6, 4, 0, 1, 2, 5, 3)      # b o core x ugrp jp half
    return np.ascontiguousarray(out.reshape(64, 64, 32, 32), dtype=np.float32)


def run_on_device(in_maps, trace=False, compute_np=None, **kwargs):
    from concourse import bass_utils

    key = ("nc", np.dtype(compute_np or COMPUTE_NP).name)
    if key not in _CACHE:
        _CACHE[key] = build_nc(compute_np)
    nc = _CACHE[key]
    res = bass_utils.run_bass_kernel_spmd(
        nc, in_maps, core_ids=list(range(NCORES)), trace=trace, **kwargs
    )
    return res


def kernel(x, weight, bias):
    in_maps = pack_inputs(x, weight, bias)
    res = run_on_device(in_maps)
    return unpack_output([r["out"] for r in res.results])


# revision 7
# speedup vs baseline: 1.2119x; 1.0431x over previous
"""LocallyConnected2d (64,64,32,32) x (1,64,64,32,32,9) -> (64,64,32,32) on 8 trn2 cores.

Strategy
--------
Spatial sharding over output rows: core i computes output rows [4i, 4i+4).

Per output location the op is an independent tiny GEMM contracting over
(c, k) = 64*9 = 576 with M = 64 out-channels, N = 64 batch. Adjacent
output columns (A=2u, B=2u+1) share two of their three input columns, so
we pack each pair into dense full-width matmuls:

  - shared taps: per input row r, one M=128 K=128 matmul.
    stationary [k=(c, col 2u+1 | c, col 2u+2), m=(A o | B o)] is 100%
    dense (A's kw1/kw2 and B's kw0/kw1); the moving operand is the
    (copy-A | copy-B) x layout read at base col 2u+1. M=128 stationaries
    are FWL-eligible (2x weight load vs the M=64 form).
  - exclusive taps (A kw0 at col 2u, B kw2 at col 2u+3): per input row,
    two concurrent M=64 K=64 col-group matmuls (tile_position (0,0) and
    (64,64)), reading copy-A at col 2u and copy-B at col 2u+2.
  - pairs at the image edge skip the exclusive matmul that would read
    the zero padding.

This cuts LDWEIGHTS columns 1.5x and matmul count 776 -> ~580 vs the
previous per-tap M=64 scheme, pushing the PE stream under the DMA wall.

DMA: all weights are SBUF-resident (16 half-bank chunks, no pool
recycling), issued upfront alternating sync/scalar HWDGE queues so the
weight stream runs at full rate instead of being throttled by PE
progress. x lower half on sync first; copy-B halves built by the DVE.
Outputs staged as fp16 (halves write traffic) and DMAed on the gpsimd
queue so they never queue behind weights. Bias is folded in with one
K=8 indicator matmul per PSUM bank. Host unpacks the device layout.

Compute dtype fp16 (fp32 accumulate in PSUM): |inputs| ~ N(0,1) so fp16
range is safe; measured rel err ~3e-4 vs fp32 reference.
"""

import numpy as np

N_B, C, H, W_W, O = 64, 64, 32, 32, 64
KH = KW = 3
NCORES = 8
RPC = H // NCORES            # 4 output rows per core
BAND = RPC + 2               # 6 padded input rows per core
WP = W_W + 2                 # 34 padded width
XFREE = BAND * WP * N_B      # 13056, layout (h, w, b) -- b innermost
XPAD = 64                    # pad so the +1-col (=+64 elem) copy-B read is in bounds
NTILE = 8                    # PSUM banks per core (8 pairs each)
PAIR_COLS = 576              # weight cols per location pair (384 shared + 192 excl)
W_FREE = NTILE * 8 * PAIR_COLS  # 36864
XH = 3 * WP * 64             # 6528: first chunk = padded rows 0-2

COMPUTE_NP = np.float16

_CACHE = {}


def _mybir_dt(np_dt):
    import concourse.mybir as mybir
    import ml_dtypes

    if np_dt == np.float16:
        return mybir.dt.float16
    if np_dt == np.float32:
        return mybir.dt.float32
    if np_dt == ml_dtypes.bfloat16:
        return mybir.dt.bfloat16
    raise ValueError(np_dt)


def build_nc(compute_np=None):
    """Build the (single-program) Bass kernel; same NEFF runs on all 8 cores."""
    import concourse.bass as bass  # noqa: F401
    import concourse.mybir as mybir
    import concourse.tile as tile
    from concourse import bacc
    from contextlib import ExitStack

    cdt = _mybir_dt(compute_np or COMPUTE_NP)
    f32 = mybir.dt.float32

    nc = bacc.Bacc("TRN2", target_bir_lowering=False, debug=False)

    x_dram = nc.dram_tensor("xb", [64, XFREE + XPAD], cdt, kind="ExternalInput")
    w_dram = nc.dram_tensor("wp", [128, W_FREE], cdt, kind="ExternalInput")
    b_dram = nc.dram_tensor("bp", [8, NTILE * 128], cdt, kind="ExternalInput")
    i_dram = nc.dram_tensor("ind", [8, 512], cdt, kind="ExternalInput")
    o_dram = nc.dram_tensor("out", [NTILE, 128, 512], cdt, kind="ExternalOutput")

    ROWF = WP * 64  # 2176 elems per padded input row

    with ExitStack() as ctx:
        tc = ctx.enter_context(tile.TileContext(nc))
        const = ctx.enter_context(tc.tile_pool(name="const", bufs=1))
        wpool = ctx.enter_context(tc.tile_pool(name="wpool", bufs=36))
        ppool = ctx.enter_context(tc.tile_pool(name="ppool", bufs=8, space="PSUM"))
        spool = ctx.enter_context(tc.tile_pool(name="spool", bufs=8))

        xsb = const.tile([128, XFREE], cdt)
        bias_sb = const.tile([8, NTILE * 128], cdt)
        ind_sb = const.tile([8, 512], cdt)

        # x lands as 6 per-row chunks: rows 0/1 on the two HWDGE rings so the
        # first matmuls aren't gated behind one queue, rows 2-5 on gpsimd.
        x_eng = [nc.sync, nc.scalar, nc.gpsimd, nc.gpsimd, nc.gpsimd, nc.gpsimd]
        for r in range(BAND):
            x_eng[r].dma_start(
                xsb[0:64, r * ROWF : (r + 1) * ROWF],
                x_dram.ap()[:, r * ROWF : (r + 1) * ROWF],
            )
        # copy-B (+1 col = +64 elems) upper halves, built per-row by the DVE.
        # col 33 of the upper half is never read, so each copy stays row-local.
        for r in range(BAND):
            nc.vector.tensor_copy(
                xsb[64:128, r * ROWF : r * ROWF + 33 * 64],
                xsb[0:64, r * ROWF + 64 : (r + 1) * ROWF],
            )
        nc.gpsimd.dma_start(bias_sb[:], b_dram.ap()[:, :])
        nc.gpsimd.dma_start(ind_sb[:], i_dram.ap()[:, :])

        # weight chunks, all issued upfront: bank 0 as 8 single-pair chunks
        # (so the first matmuls start early), banks 1-7 as 2-pair chunks.
        # Spread across sync/scalar HWDGE rings plus a slice on gpsimd.
        chunk_of = {}   # (t, jp) -> tile index
        chunks = []     # (t, pair_lo, npairs)
        for jp in range(8):
            chunk_of[(0, jp)] = len(chunks)
            chunks.append((0, jp, 1))
        for t in range(1, NTILE):
            for s in range(4):
                for jp in (2 * s, 2 * s + 1):
                    chunk_of[(t, jp)] = len(chunks)
                chunks.append((t, 2 * s, 2))
        # q mod 2 alternates sync/scalar; a few mid/late chunks go to gpsimd
        GPS_CHUNKS = {14, 19, 24, 29, 34}
        wts = []
        for q, (t, plo, np_) in enumerate(chunks):
            wt = wpool.tile([128, np_ * PAIR_COLS], cdt)
            off = (t * 8 + plo) * PAIR_COLS
            if q in GPS_CHUNKS:
                eng = nc.gpsimd
            else:
                eng = nc.sync if q % 2 == 0 else nc.scalar
            eng.dma_start(wt[:], w_dram.ap()[:, off : off + np_ * PAIR_COLS])
            wts.append(wt)

        for t in range(NTILE):
            x_row = t // 2
            ps = ppool.tile([128, 512], f32)
            first = True
            for jp in range(8):
                wt = wts[chunk_of[(t, jp)]]
                base = (jp - chunks[chunk_of[(t, jp)]][1]) * PAIR_COLS
                u = (t % 2) * 8 + jp          # global pair index in this x-row
                oc = jp * 64
                # shared taps: M=128 K=128, moving = (copyA|copyB) at col 2u+1
                for r in range(3):
                    fo = ((x_row + r) * WP + 2 * u + 1) * 64
                    nc.tensor.matmul(
                        ps[:, oc : oc + 64],
                        wt[:, base + r * 128 : base + (r + 1) * 128],
                        xsb[:, fo : fo + 64],
                        start=first,
                        stop=False,
                        skip_group_check=True,
                    )
                    first = False
                # exclusive taps: concurrent M=64 K=64 col-group matmuls
                for r in range(3):
                    eb = base + 384 + r * 64
                    if u > 0:  # A kw0 reads col 2u (zero pad when u==0)
                        foA = ((x_row + r) * WP + 2 * u) * 64
                        nc.tensor.matmul(
                            ps[0:64, oc : oc + 64],
                            wt[0:64, eb : eb + 64],
                            xsb[0:64, foA : foA + 64],
                            start=False,
                            stop=False,
                            skip_group_check=True,
                        )
                    if u < 15:  # B kw2 reads col 2u+3 (zero pad when u==15)
                        foB = ((x_row + r) * WP + 2 * u + 2) * 64
                        nc.tensor.matmul(
                            ps[64:128, oc : oc + 64],
                            wt[64:128, eb : eb + 64],
                            xsb[64:128, foB : foB + 64],
                            start=False,
                            stop=False,
                            skip_group_check=True,
                        )
            # bias: psum[p, j*64+b] += bias[j, t*128+p] * ind[j, col]
            nc.tensor.matmul(
                ps[:, :],
                bias_sb[:, t * 128 : (t + 1) * 128],
                ind_sb[:, :],
                start=False,
                stop=True,
                skip_group_check=True,
            )
            stg = spool.tile([128, 512], cdt)
            nc.scalar.copy(stg[:], ps[:])
            # last bank's output goes on sync: its HWDGE ring is empty by
            # then, so the final store isn't stuck behind SWDGE completion
            out_eng = nc.sync if t == NTILE - 1 else nc.gpsimd
            out_eng.dma_start(o_dram.ap()[t], stg[:])

    nc.compile()
    return nc


def pack_inputs(x, weight, bias, compute_np=None):
    """Full fp32 inputs -> list of 8 per-core input dicts (device layouts)."""
    cnp = compute_np or COMPUTE_NP
    x = np.asarray(x)
    w5 = np.asarray(weight)[0]        # (o, c, X, Y, k)
    b3 = np.asarray(bias)[0]          # (o, X, Y)

    xp = np.pad(x, ((0, 0), (0, 0), (1, 1), (1, 1)))  # (b, c, 34, 34)

    ind = np.zeros((8, 512), dtype=cnp)
    for j in range(8):
        ind[j, j * 64 : (j + 1) * 64] = 1.0

    in_maps = []
    for i in range(NCORES):
        band = xp[:, :, RPC * i : RPC * i + BAND, :]          # (b, c, 6, 34)
        xb = np.ascontiguousarray(band.transpose(1, 2, 3, 0)) # (c, 6, 34, b)
        xb = xb.astype(cnp).reshape(64, XFREE)
        xb = np.concatenate([xb, np.zeros((64, XPAD), dtype=cnp)], axis=1)

        wc = w5[:, :, RPC * i : RPC * (i + 1), :, :]          # (o, c, 4, 32, 9)
        # (x, u, c, ab, k, o)
        wt_ = wc.reshape(64, 64, 4, 16, 2, 9).transpose(2, 3, 1, 4, 5, 0)
        # shared stationary: [x, u, r, p2(c-half: col 2u+1 / 2u+2), c, m2(A/B), o]
        # col 2u+1 is A kw1 / B kw0; col 2u+2 is A kw2 / B kw1
        KWT = ((1, 0), (2, 1))
        S = np.empty((4, 16, 3, 2, 64, 2, 64), dtype=np.float32)
        for r in range(3):
            for p2 in range(2):
                for m2 in range(2):
                    S[:, :, r, p2, :, m2, :] = wt_[:, :, :, m2, 3 * r + KWT[p2][m2], :]
        # exclusive stationary: [x, u, r, half(eA/eB), c, o]
        E = np.empty((4, 16, 3, 2, 64, 64), dtype=np.float32)
        for r in range(3):
            E[:, :, r, 0] = wt_[:, :, :, 0, 3 * r + 0, :]     # A kw0
            E[:, :, r, 1] = wt_[:, :, :, 1, 3 * r + 2, :]     # B kw2
        # per pair: cols = [r, m2, o] (384 shared) then [r, o] (192 excl),
        # partitions = (p2|half, c)
        Sp = S.transpose(0, 1, 3, 4, 2, 5, 6).reshape(4, 16, 128, 384)
        Ep = E.transpose(0, 1, 3, 4, 2, 5).reshape(4, 16, 128, 192)
        wq = np.concatenate([Sp, Ep], axis=3).reshape(4, 2, 8, 128, PAIR_COLS)
        wq = wq.transpose(3, 0, 1, 2, 4)                      # (p, x, ugrp, jp, cols)
        wq = np.ascontiguousarray(wq).astype(cnp).reshape(128, W_FREE)

        bc = b3[:, RPC * i : RPC * (i + 1), :]                # (o, 4, 32)
        bcr = bc.reshape(64, 4, 16, 2)                        # o x u half
        bcr = bcr.reshape(64, 4, 2, 8, 2)                     # o x ugrp j half
        # bp[j, (x, ugrp), half, o]
        bp = bcr.transpose(3, 1, 2, 4, 0).reshape(8, NTILE * 128).astype(cnp)

        in_maps.append(
            {
                "xb": xb,
                "wp": wq,
                "bp": np.ascontiguousarray(bp),
                "ind": ind,
            }
        )
    return in_maps


def unpack_output(core_outs):
    """8 per-core [NTILE,128,512] arrays -> full (64, 64, 32, 32) output."""
    arr = np.stack([np.asarray(a, dtype=np.float32) for a in core_outs])
    arr = arr.reshape(8, 4, 2, 2, 64, 8, 64)      # core x ugrp half o jp b
    out = arr.transpose(6, 4, 0, 1, 2, 5, 3)      # b o core x ugrp jp half
    return np.ascontiguousarray(out.reshape(64, 64, 32, 32), dtype=np.float32)


def run_on_device(in_maps, trace=False, compute_np=None, **kwargs):
    from concourse import bass_utils

    key = ("nc", np.dtype(compute_np or COMPUTE_NP).name)
    if key not in _CACHE:
        _CACHE[key] = build_nc(compute_np)
    nc = _CACHE[key]
    res = bass_utils.run_bass_kernel_spmd(
        nc, in_maps, core_ids=list(range(NCORES)), trace=trace, **kwargs
    )
    return res


def kernel(x, weight, bias):
    in_maps = pack_inputs(x, weight, bias)
    res = run_on_device(in_maps)
    return unpack_output([r["out"] for r in res.results])
